# revision 2
# baseline (speedup 1.0000x reference)
"""Trainium2 Bass kernel for nn_CapacitanceMatrix.

C[b, i, j] = sigmoid(x[b]·Wd[i] + bd[i])        if i == j
           = -softplus(x[b]·Wo[m] + bo[m])      if i != j  (m = row-major off-diag idx)

v2 design (baseline 81.4us profile -> target ~72us):
- x is relaid out on the host into chunk-interleaved "window" tiles:
  xT[w] = [128 partitions, 8 chunks x 512 cols], so ONE 1MB DMA (128 fully
  contiguous 8KB rows) delivers all 8 contraction chunks for 512 batch
  columns. 16 windows per core; the first two are processed as single-window
  segments so the first epilogue fires ~2us earlier.
- Whole x working set streams through an 8-deep pool (64KB/partition); all
  triggers are issued up-front on the sync HWDGE ring with zero cross-engine
  dependencies (weights + bias first, then the 16 windows, then the output
  DMAs which wait in-queue on their scalar-produced tiles — by construction
  those waits are satisfied long before the queue reaches them).
- Weights are one [128, 8*256] tile (single 0.5MB DMA). lhsT(c, half) is a
  128-col slice; the 256 fused rows are [240 off-diag | 16 negated diag].
- Epilogue per segment-half on ScalarE only (Exp -> Ln; diag rows get a
  third Exp on the 32-aligned [96:128] window into a scratch tile):
    ev = exp(psum + b); sp = ln(1 + ev)          (softplus, bf16)
    diag (psum held -z): sigmoid(z) = exp(-sp)
  Off-diag ships as +softplus; the host negates during the f32 gather.
- Output DMAs ride the same sync HWDGE queue (the baseline used the gpsimd
  SWDGE path at ~69 GB/s, which dragged a ~11us half-clock tail).
- DVFS: the PE drops to half clock unless kept continuously busy (~9us
  ramp observed). 32 back-to-back throwaway matmuls bridge the preamble
  from PE-ready (~3.4us) to first-input-landed (~12us). A dummy Exp right
  after engine start pulls the 1.28us ACT_TABLE_LOAD off the first
  epilogue's critical path.
"""

import sys

sys.path.insert(0, "/opt/trn_rl_repo")

from contextlib import ExitStack

import numpy as np

import concourse.bass as bass  # noqa: F401  (kept for parity with framework imports)
import concourse.tile as tile
from concourse import bacc, mybir
from concourse.bass_utils import run_bass_kernel_spmd

B = 65536
D = 1024
K = 16
NOUT = K * K  # 256
NCORES = 8
BC = B // NCORES  # 8192 rows per core
KD = D // 128  # 8 contraction chunks
NW = BC // 512  # 16 windows of 512 batch cols
# window groups per epilogue segment: first two fire early, rest pairwise
SEGS = [[0], [1]] + [[w, w + 1] for w in range(2, NW, 2)]
NDIAG_P0 = 240 - 128  # partition where diag rows start in half B (112)
N_WARMUP = 32

MM_DT_NAME = "bfloat16"  # kept for test.py compat

_CACHE = {}

_ACT_TABLES_PATCHED = False


def _pin_act_table_set():
    """Force Exp and Ln to resolve to the single LUT set that holds both
    (`natural_log_exp_and_others`) so the exp->ln alternation never thrashes
    ACT_TABLE_LOADs."""
    global _ACT_TABLES_PATCHED
    if _ACT_TABLES_PATCHED:
        return
    import concourse.hw_specs as hw_specs

    orig = hw_specs.get_activation_tables

    def patched(arch):
        tables = {k: set(v) for k, v in orig(arch).items()}
        keep = "natural_log_exp_and_others"
        if keep in tables:
            for k, v in tables.items():
                if k != keep:
                    v.discard(mybir.ActivationFunctionType.Exp)
                    v.discard(mybir.ActivationFunctionType.Ln)
        return tables

    bacc.get_activation_tables = patched
    _ACT_TABLES_PATCHED = True


def _build_bass():
    _pin_act_table_set()
    bf16 = mybir.dt.bfloat16
    f32 = mybir.dt.float32
    nc = bacc.Bacc("TRN2", target_bir_lowering=False, debug=False)
    # window tiles: xT[w, p, c*512 + t] = x[w*512 + t, c*128 + p]
    xT = nc.dram_tensor("xT", [NW, 128, KD * 512], bf16, kind="ExternalInput").ap()
    # wts[p, c*256 + n] = W_dev[n, c*128 + p]; W_dev = [Wo; -Wd]
    wts = nc.dram_tensor("wts", [128, KD * NOUT], bf16, kind="ExternalInput").ap()
    # biasv[p, h] = bias for output row h*128+p ([bo; -bd])
    biasv = nc.dram_tensor("biasv", [128, 2], f32, kind="ExternalInput").ap()
    # transposed output: row r = fused weight row, col = batch index in core
    outT = nc.dram_tensor("outT", [NOUT, BC], bf16, kind="ExternalOutput").ap()

    with tile.TileContext(nc) as tc, ExitStack() as ctx:
        const_pool = ctx.enter_context(tc.tile_pool(name="const", bufs=1))
        x_pool = ctx.enter_context(tc.tile_pool(name="x", bufs=8))
        ev_pool = ctx.enter_context(tc.tile_pool(name="ev", bufs=4))
        # sp/sc live until their output DMA drains at the back of the sync
        # queue; one buffer per allocation means zero recycling stalls
        sp_pool = ctx.enter_context(tc.tile_pool(name="sp", bufs=2 * len(SEGS)))
        sc_pool = ctx.enter_context(tc.tile_pool(name="sc", bufs=len(SEGS)))
        psum_pool = ctx.enter_context(tc.tile_pool(name="ps", bufs=4, space="PSUM"))

        # sync HWDGE carries everything: weights+bias first, then the 16 x
        # windows, with output DMAs interleaved later in queue order
        wt = const_pool.tile([128, KD * NOUT], bf16, tag="wt")
        nc.sync.dma_start(wt[:], wts)
        bias_sb = const_pool.tile([128, 2], f32, tag="bias")
        nc.sync.dma_start(bias_sb[:], biasv)
        xw = []
        for w in range(NW):
            xt = x_pool.tile([128, KD * 512], bf16, tag="x")
            nc.sync.dma_start(xt[:], xT[w])
            xw.append(xt)

        # warm the PE's DVFS p-state with throwaway matmuls bridging the
        # preamble until the first x window lands; an idle PE runs at ~half
        # clock for its first ~9us of work otherwise
        dum_sb = const_pool.tile([128, 512], bf16, tag="dum")
        nc.vector.memset(dum_sb[:], 0.0)
        ps_dum = psum_pool.tile([128, 1024], f32, tag="ps")
        for _ in range(N_WARMUP):
            nc.tensor.matmul(
                ps_dum[0:16, 0:512],
                lhsT=dum_sb[:, 0:16],
                rhs=dum_sb[:],
                start=True,
                stop=True,
                skip_group_check=True,
            )

        # pull the 1.28us ACT_TABLE_LOAD off the first epilogue's critical
        # path: a dummy Exp while the scalar engine is otherwise idle
        tiny = const_pool.tile([128, 8], bf16, tag="tiny")
        nc.scalar.activation(tiny[:], dum_sb[:, 0:8], mybir.ActivationFunctionType.Exp)

        for ws in SEGS:
            cw = 512 * len(ws)
            col0 = ws[0] * 512
            cols = slice(col0, col0 + cw)
            # half B (3-deep ScalarE chain) first, half A (2-deep) last so
            # the drain after the final matmul is as short as possible
            for half in (1, 0):
                ps = psum_pool.tile([128, 1024], f32, tag="ps")
                for c in range(KD):
                    lhsT = wt[:, c * NOUT + half * 128 : c * NOUT + half * 128 + 128]
                    for wi, w in enumerate(ws):
                        nc.tensor.matmul(
                            ps[:, wi * 512 : (wi + 1) * 512],
                            lhsT=lhsT,
                            rhs=xw[w][:, c * 512 : (c + 1) * 512],
                            start=(c == 0),
                            stop=(c == KD - 1),
                            skip_group_check=True,
                        )
                # off-diag rows ship as +softplus (the host negates during
                # the f32 gather), so the DVE never touches the data
                ev = ev_pool.tile([128, 1024], bf16, tag="ev")
                nc.scalar.activation(
                    ev[:, :cw],
                    ps[:, :cw],
                    mybir.ActivationFunctionType.Exp,
                    bias=bias_sb[:, half : half + 1],
                )
                sp = sp_pool.tile([128, 1024], bf16, tag="sp")
                nc.scalar.activation(
                    sp[:, :cw], ev[:, :cw], mybir.ActivationFunctionType.Ln, bias=1.0
                )
                if half == 0:
                    nc.sync.dma_start(outT[0:128, cols], sp[:, :cw])
                else:
                    # diag rows sit at [112:128] where psum held -z, so
                    # sp = ln(1+e^-z) = softplus(-z) there and
                    # sigmoid(z) = exp(-sp). Compute-engine APs need
                    # 32-aligned partition starts, so the diag Exp runs
                    # on [96:128] into a scratch tile (96..111 junk never
                    # DMA'd out; DMA APs have no alignment limit).
                    p0 = NDIAG_P0  # 112
                    sc = sc_pool.tile([128, 1024], bf16, tag="sc")
                    nc.scalar.activation(
                        sc[96:128, :cw],
                        sp[96:128, :cw],
                        mybir.ActivationFunctionType.Exp,
                        scale=-1.0,
                    )
                    nc.sync.dma_start(outT[128 : 128 + p0, cols], sp[0:p0, :cw])
                    nc.sync.dma_start(outT[128 + p0 : 256, cols], sc[p0:128, :cw])
    nc.compile()
    return nc


def _get_nc():
    if "nc" not in _CACHE:
        _CACHE["nc"] = _build_bass()
    return _CACHE["nc"]


def _host_prep(x, Wd, bd, Wo, bo):
    import ml_dtypes

    np_bf16 = ml_dtypes.bfloat16
    # fused rows: [Wo (240) ; -Wd (16)] — diag negated so psum holds -z and
    # sigmoid(z) = 1/(1 + e^-z) comes out of the shared exp pass
    w_dev = np.concatenate([Wo, -Wd], axis=0)  # (256, D)
    b_dev = np.concatenate([bo, -bd], axis=0)  # (256,)
    wts = np.ascontiguousarray(
        w_dev.T.reshape(KD, 128, NOUT).transpose(1, 0, 2).reshape(128, KD * NOUT)
    ).astype(np_bf16)
    biasv = np.ascontiguousarray(
        np.stack([b_dev[0:128], b_dev[128:256]], axis=1)
    ).astype(np.float32)
    in_maps = []
    for c in range(NCORES):
        xs = x[c * BC : (c + 1) * BC]  # (BC, D)
        # -> (NW, 128, KD*512): elem (w, p, c*512+t) = xs[w*512+t, c*128+p]
        xTc = np.ascontiguousarray(
            xs.reshape(NW, 512, KD, 128).transpose(0, 3, 2, 1).reshape(NW, 128, KD * 512)
        ).astype(np_bf16)
        in_maps.append({"xT": xTc, "wts": wts, "biasv": biasv})
    return in_maps


def _install_env_shims():
    """The agent image's `antenv` stub lacks `axon_hooks`; bass_utils imports
    it on any trace=True/BASS_TRACE run. Provide it (wired to the ctypes NTFF
    hook when available), and skip the S3 artifact upload (no egress)."""
    if "antenv.axon_hooks" in sys.modules:
        return
    import types

    try:
        import antenv
    except ImportError:
        return
    if hasattr(antenv, "axon_hooks"):
        return
    mod = types.ModuleType("antenv.axon_hooks")
    hook = [None]
    try:
        from trn_agent_boot.trn_boot import _ntff_profile_via_ctypes

        hook[0] = _ntff_profile_via_ctypes("/opt/axon/libaxon_pjrt.so")
    except Exception:
        pass
    mod.set_axon_ntff_profile_hook = lambda h: hook.__setitem__(0, h)
    mod.get_axon_ntff_profile_hook = lambda: hook[0]
    sys.modules["antenv.axon_hooks"] = mod
    antenv.axon_hooks = mod

    import concourse.bass_utils as bu

    bu.upload_artifacts = lambda tmpdir: tmpdir


def _run(in_maps, **kwargs):
    _install_env_shims()
    nc = _get_nc()
    return run_bass_kernel_spmd(nc, in_maps, list(range(NCORES)), **kwargs)


# row r of outT -> flat (i, j) position: P[i*16+j] = source row
def _out_perm():
    off_i, off_j = np.nonzero(~np.eye(K, dtype=bool))
    P = np.empty(NOUT, np.int64)
    P[off_i * K + off_j] = np.arange(K * (K - 1))
    P[np.arange(K) * (K + 1)] = K * (K - 1) + np.arange(K)
    return P


def kernel(x, Wd, bd, Wo, bo, _bench_results=None, **kwargs):
    x = np.asarray(x, np.float32)
    in_maps = _host_prep(
        x,
        np.asarray(Wd, np.float32),
        np.asarray(bd, np.float32),
        np.asarray(Wo, np.float32),
        np.asarray(bo, np.float32),
    )
    res = _run(in_maps, **kwargs)
    if _bench_results is not None:
        _bench_results.append(res)
    P = _out_perm()
    out = np.empty((B, NOUT), np.float32)
    for c in range(NCORES):
        oT = np.asarray(res.results[c]["outT"], dtype=np.float32)  # (256, BC)
        # device ships +softplus for the 240 off-diag rows; negate here
        oT[: K * (K - 1)] *= -1.0
        out[c * BC : (c + 1) * BC] = oT[P].T
    return out.reshape(B, K, K)


# revision 3
# speedup vs baseline: 1.0127x; 1.0127x over previous
"""Trainium2 Bass kernel for nn_CapacitanceMatrix.

C[b, i, j] = sigmoid(x[b]·Wd[i] + bd[i])        if i == j
           = -softplus(x[b]·Wo[m] + bo[m])      if i != j  (m = row-major off-diag idx)

v3 design:
- x is relaid out on the host into chunk-interleaved "window" tiles:
  xT[w] = [128 partitions, 8 chunks x 512 cols], so ONE 1MB DMA (128 fully
  contiguous 8KB rows) delivers all 8 contraction chunks for 512 batch
  columns. 16 windows per core, all resident in SBUF (no pool recycling),
  so all input triggers fire back-to-back on the sync HWDGE ring with zero
  dependencies. The first two windows are their own epilogue segments so
  the first psum completes ~2us earlier; the rest pair up into 1024-col
  segments.
- Weights are one [128, 8*256] tile (single 0.5MB DMA). lhsT(c, half) is a
  128-col slice; the 256 fused rows are [240 off-diag | 16 negated diag].
- Epilogue per segment on ScalarE only:
    ev = exp(psum + b); sp = ln(1 + ev)          (softplus, bf16)
    diag (psum held -z): sigmoid(z) = exp(-sp)
  Mid-stream segments run both psum-freeing Exps first (ExpB, ExpA, LnB,
  scExp, LnA) so the PE's psum rotation never waits on the scalar chain;
  the last segment keeps the B-chain-then-A-chain order for the shortest
  post-matmul drain.
- Off-diag outputs ship as +softplus via gpsimd SWDGE mid-stream (spread
  over the whole kernel, off the input queue); the host negates during the
  f32 gather. Diag sigmoid rows collect in one persistent [128, BC] tile
  and leave as a single end-of-kernel DMA. The last segment's two softplus
  DMAs ride the sync HWDGE queue, which is empty by then — this keeps the
  drain off the ~1us-per-DMA SWDGE generation path.
- DVFS: the PE drops to ~half clock unless kept continuously busy (~6-9us
  ramp). 10 back-to-back throwaway matmuls bridge the preamble from
  PE-ready to first-input-landed. A dummy Exp right after engine start
  pulls the 1.28us ACT_TABLE_LOAD off the first epilogue's critical path.
"""

import sys

sys.path.insert(0, "/opt/trn_rl_repo")

from contextlib import ExitStack

import numpy as np

import concourse.bass as bass  # noqa: F401  (kept for parity with framework imports)
import concourse.tile as tile
from concourse import bacc, mybir
from concourse.bass_utils import run_bass_kernel_spmd

B = 65536
D = 1024
K = 16
NOUT = K * K  # 256
NCORES = 8
BC = B // NCORES  # 8192 rows per core
KD = D // 128  # 8 contraction chunks
NW = BC // 512  # 16 windows of 512 batch cols
# window groups per epilogue segment: first two fire early, rest pairwise
SEGS = [[0], [1]] + [[w, w + 1] for w in range(2, NW, 2)]
NDIAG_P0 = 240 - 128  # partition where diag rows start in half B (112)
N_WARMUP = 10

MM_DT_NAME = "bfloat16"  # kept for test.py compat

_CACHE = {}

_ACT_TABLES_PATCHED = False


def _pin_act_table_set():
    """Force Exp and Ln to resolve to the single LUT set that holds both
    (`natural_log_exp_and_others`) so the exp->ln alternation never thrashes
    ACT_TABLE_LOADs."""
    global _ACT_TABLES_PATCHED
    if _ACT_TABLES_PATCHED:
        return
    import concourse.hw_specs as hw_specs

    orig = hw_specs.get_activation_tables

    def patched(arch):
        tables = {k: set(v) for k, v in orig(arch).items()}
        keep = "natural_log_exp_and_others"
        if keep in tables:
            for k, v in tables.items():
                if k != keep:
                    v.discard(mybir.ActivationFunctionType.Exp)
                    v.discard(mybir.ActivationFunctionType.Ln)
        return tables

    bacc.get_activation_tables = patched
    _ACT_TABLES_PATCHED = True


def _build_bass():
    _pin_act_table_set()
    bf16 = mybir.dt.bfloat16
    f32 = mybir.dt.float32
    Exp = mybir.ActivationFunctionType.Exp
    Ln = mybir.ActivationFunctionType.Ln
    nc = bacc.Bacc("TRN2", target_bir_lowering=False, debug=False)
    # window tiles: xT[w, p, c*512 + t] = x[w*512 + t, c*128 + p]
    xT = nc.dram_tensor("xT", [NW, 128, KD * 512], bf16, kind="ExternalInput").ap()
    # wts[p, c*256 + n] = W_dev[n, c*128 + p]; W_dev = [Wo; -Wd]
    wts = nc.dram_tensor("wts", [128, KD * NOUT], bf16, kind="ExternalInput").ap()
    # biasv[p, h] = bias for output row h*128+p ([bo; -bd])
    biasv = nc.dram_tensor("biasv", [128, 2], f32, kind="ExternalInput").ap()
    # transposed output: row r = fused weight row, col = batch index in core
    outT = nc.dram_tensor("outT", [NOUT, BC], bf16, kind="ExternalOutput").ap()

    with tile.TileContext(nc) as tc, ExitStack() as ctx:
        const_pool = ctx.enter_context(tc.tile_pool(name="const", bufs=1))
        x_pool = ctx.enter_context(tc.tile_pool(name="x", bufs=NW))
        ev_pool = ctx.enter_context(tc.tile_pool(name="ev", bufs=4))
        sp_pool = ctx.enter_context(tc.tile_pool(name="sp", bufs=6))
        psum_pool = ctx.enter_context(tc.tile_pool(name="ps", bufs=4, space="PSUM"))

        # sync HWDGE carries the input stream: weights+bias, then all 16 x
        # windows back-to-back (whole x is SBUF-resident; no WAR waits)
        wt = const_pool.tile([128, KD * NOUT], bf16, tag="wt")
        nc.sync.dma_start(wt[:], wts)
        bias_sb = const_pool.tile([128, 2], f32, tag="bias")
        nc.sync.dma_start(bias_sb[:], biasv)
        xw = []
        for w in range(NW):
            xt = x_pool.tile([128, KD * 512], bf16, tag="x")
            nc.sync.dma_start(xt[:], xT[w])
            xw.append(xt)

        # warm the PE's DVFS p-state with throwaway matmuls bridging the
        # preamble until the first x window lands; an idle PE runs at ~half
        # clock for its first several us of work otherwise
        dum_sb = const_pool.tile([128, 512], bf16, tag="dum")
        nc.vector.memset(dum_sb[:], 0.0)
        ps_dum = psum_pool.tile([128, 1024], f32, tag="ps")
        for _ in range(N_WARMUP):
            nc.tensor.matmul(
                ps_dum[0:16, 0:512],
                lhsT=dum_sb[:, 0:16],
                rhs=dum_sb[:],
                start=True,
                stop=True,
                skip_group_check=True,
            )

        # pull the 1.28us ACT_TABLE_LOAD off the first epilogue's critical
        # path: a dummy Exp while the scalar engine is otherwise idle
        tiny = const_pool.tile([128, 8], bf16, tag="tiny")
        nc.scalar.activation(tiny[:], dum_sb[:, 0:8], Exp)

        # all diag sigmoid rows collect here; one DMA ships them at the end
        sc_all = const_pool.tile([128, BC], bf16, tag="sc")

        p0 = NDIAG_P0  # 112
        nseg = len(SEGS)
        for si, ws in enumerate(SEGS):
            last = si == nseg - 1
            cw = 512 * len(ws)
            col0 = ws[0] * 512
            cols = slice(col0, col0 + cw)
            # half B (rows 128..255, incl. diag) first, half A last
            pss = {}
            for half in (1, 0):
                ps = psum_pool.tile([128, 1024], f32, tag="ps")
                pss[half] = ps
                for c in range(KD):
                    lhsT = wt[:, c * NOUT + half * 128 : c * NOUT + half * 128 + 128]
                    for wi, w in enumerate(ws):
                        nc.tensor.matmul(
                            ps[:, wi * 512 : (wi + 1) * 512],
                            lhsT=lhsT,
                            rhs=xw[w][:, c * 512 : (c + 1) * 512],
                            start=(c == 0),
                            stop=(c == KD - 1),
                            skip_group_check=True,
                        )
            # scalar epilogue. ev = exp(psum + b); sp = ln(1+ev) = softplus.
            # Diag rows sit at [112:128] of half B where psum held -z, so
            # sp = softplus(-z) and sigmoid(z) = exp(-sp). Compute-engine APs
            # need 32-aligned partition starts, so the diag Exp runs on
            # [96:128] into sc_all (96..111 junk never DMA'd out).
            # Mid-stream: both psum-freeing Exps run first so the PE's psum
            # rotation never waits on the scalar chain.
            evs, sps = {}, {}

            def _exp(half):
                ev = ev_pool.tile([128, 1024], bf16, tag="ev")
                nc.scalar.activation(
                    ev[:, :cw],
                    pss[half][:, :cw],
                    Exp,
                    bias=bias_sb[:, half : half + 1],
                )
                evs[half] = ev

            def _ln(half):
                sp = sp_pool.tile([128, 1024], bf16, tag="sp")
                nc.scalar.activation(
                    sp[:, :cw], evs[half][:, :cw], Ln, bias=1.0
                )
                sps[half] = sp

            def _scexp():
                nc.scalar.activation(
                    sc_all[96:128, cols],
                    sps[1][96:128, :cw],
                    Exp,
                    scale=-1.0,
                )

            if not last:
                _exp(1); _exp(0); _ln(1); _scexp(); _ln(0)
                # off-diag softplus leaves via gpsimd SWDGE, spread over the
                # whole kernel and off the input queue
                nc.gpsimd.dma_start(outT[128 : 128 + p0, cols], sps[1][0:p0, :cw])
                nc.gpsimd.dma_start(outT[0:128, cols], sps[0][:, :cw])
            else:
                # shortest possible post-matmul drain: B chain, then A chain,
                # with the DMAs on the (by now idle) sync HWDGE queue
                _exp(1); _ln(1); _scexp()
                nc.sync.dma_start(outT[128 : 128 + p0, cols], sps[1][0:p0, :cw])
                _exp(0); _ln(0)
                nc.sync.dma_start(outT[0:128, cols], sps[0][:, :cw])
        # single DMA for all diag sigmoid rows
        nc.sync.dma_start(outT[128 + p0 : 256, :], sc_all[p0:128, :])
    nc.compile()
    return nc


def _get_nc():
    if "nc" not in _CACHE:
        _CACHE["nc"] = _build_bass()
    return _CACHE["nc"]


def _host_prep(x, Wd, bd, Wo, bo):
    import ml_dtypes

    np_bf16 = ml_dtypes.bfloat16
    # fused rows: [Wo (240) ; -Wd (16)] — diag negated so psum holds -z and
    # sigmoid(z) = 1/(1 + e^-z) comes out of the shared exp pass
    w_dev = np.concatenate([Wo, -Wd], axis=0)  # (256, D)
    b_dev = np.concatenate([bo, -bd], axis=0)  # (256,)
    wts = np.ascontiguousarray(
        w_dev.T.reshape(KD, 128, NOUT).transpose(1, 0, 2).reshape(128, KD * NOUT)
    ).astype(np_bf16)
    biasv = np.ascontiguousarray(
        np.stack([b_dev[0:128], b_dev[128:256]], axis=1)
    ).astype(np.float32)
    in_maps = []
    for c in range(NCORES):
        xs = x[c * BC : (c + 1) * BC]  # (BC, D)
        # -> (NW, 128, KD*512): elem (w, p, c*512+t) = xs[w*512+t, c*128+p]
        xTc = np.ascontiguousarray(
            xs.reshape(NW, 512, KD, 128).transpose(0, 3, 2, 1).reshape(NW, 128, KD * 512)
        ).astype(np_bf16)
        in_maps.append({"xT": xTc, "wts": wts, "biasv": biasv})
    return in_maps


def _install_env_shims():
    """The agent image's `antenv` stub lacks `axon_hooks`; bass_utils imports
    it on any trace=True/BASS_TRACE run. Provide it (wired to the ctypes NTFF
    hook when available), and skip the S3 artifact upload (no egress)."""
    if "antenv.axon_hooks" in sys.modules:
        return
    import types

    try:
        import antenv
    except ImportError:
        return
    if hasattr(antenv, "axon_hooks"):
        return
    mod = types.ModuleType("antenv.axon_hooks")
    hook = [None]
    try:
        from trn_agent_boot.trn_boot import _ntff_profile_via_ctypes

        hook[0] = _ntff_profile_via_ctypes("/opt/axon/libaxon_pjrt.so")
    except Exception:
        pass
    mod.set_axon_ntff_profile_hook = lambda h: hook.__setitem__(0, h)
    mod.get_axon_ntff_profile_hook = lambda: hook[0]
    sys.modules["antenv.axon_hooks"] = mod
    antenv.axon_hooks = mod

    import concourse.bass_utils as bu

    bu.upload_artifacts = lambda tmpdir: tmpdir


def _run(in_maps, **kwargs):
    _install_env_shims()
    nc = _get_nc()
    return run_bass_kernel_spmd(nc, in_maps, list(range(NCORES)), **kwargs)


# row r of outT -> flat (i, j) position: P[i*16+j] = source row
def _out_perm():
    off_i, off_j = np.nonzero(~np.eye(K, dtype=bool))
    P = np.empty(NOUT, np.int64)
    P[off_i * K + off_j] = np.arange(K * (K - 1))
    P[np.arange(K) * (K + 1)] = K * (K - 1) + np.arange(K)
    return P


def kernel(x, Wd, bd, Wo, bo, _bench_results=None, **kwargs):
    x = np.asarray(x, np.float32)
    in_maps = _host_prep(
        x,
        np.asarray(Wd, np.float32),
        np.asarray(bd, np.float32),
        np.asarray(Wo, np.float32),
        np.asarray(bo, np.float32),
    )
    res = _run(in_maps, **kwargs)
    if _bench_results is not None:
        _bench_results.append(res)
    P = _out_perm()
    out = np.empty((B, NOUT), np.float32)
    for c in range(NCORES):
        oT = np.asarray(res.results[c]["outT"], dtype=np.float32)  # (256, BC)
        # device ships +softplus for the 240 off-diag rows; negate here
        oT[: K * (K - 1)] *= -1.0
        out[c * BC : (c + 1) * BC] = oT[P].T
    return out.reshape(B, K, K)
